# revision 9
# baseline (speedup 1.0000x reference)
"""Trainium2 Bass kernel for CholeskyPlusConst MLP.

Computes, for x [B, 12]:
    h1  = relu(x @ W1 + b1)            [B, 512]
    h2  = relu(h1 @ W2 + b2)           [B, 512]
    out = softplus(h2 @ W3 + b3)       [B, 79]
    L   = tril-scatter(out[:, :78])    [B, 12, 12]
    MMT = L @ L^T                      [B, 12, 12]
    c   = out[:, 78]                   [B]
returns (MMT, c).

Strategy: pure data parallel over 8 NeuronCores (batch sharded, weights
replicated). On-core the pipeline runs feature-major (features on SBUF
partitions, batch on the free dim, 512-wide batch chunks) so every layer
is a plain stationary-weight matmul. The quadratic L@L^T step uses
a*b = ((a+b)^2 - a^2 - b^2)/2: a constant 0/1 pre-map M1 (78 -> 364) on
the tensor engine, an elementwise square, and a constant post-map
M2 (364 -> 144) whose matmul takes the squared data as the *stationary*
operand so the result lands batch-major [128b, 144] and can be DMA'd out
contiguously with no transpose.
"""

import numpy as np
from contextlib import ExitStack

import concourse.bass as bass
import concourse.mybir as mybir
import concourse.tile as tile
from concourse import bacc
from concourse.bass_utils import run_bass_kernel_spmd

N = 12
TRI = 78            # N*(N+1)//2
HID = 512
NPRE = 286 + TRI    # 364 pre-square rows
B_FULL = 262144
N_CORES = 8
CHUNK = 512         # batch columns per pipeline chunk (one PSUM bank fp32)

F32 = mybir.dt.float32


def _tri_idx(i, j):
    return i * (i + 1) // 2 + j


def make_quad_maps():
    """M1 [78, 364] pre-square map and M2 [364, 144] post-square map with
    MMT.reshape(B,144) == ((tri @ M1)**2) @ M2."""
    triples = [(i, k, j) for i in range(N) for k in range(i) for j in range(k + 1)]
    M1 = np.zeros((TRI, NPRE), np.float32)
    for m, (i, k, j) in enumerate(triples):
        M1[_tri_idx(i, j), m] += 1
        M1[_tri_idx(k, j), m] += 1
    for t in range(TRI):
        M1[t, 286 + t] = 1
    M2 = np.zeros((NPRE, 144), np.float32)
    for m, (i, k, j) in enumerate(triples):
        M2[m, i * N + k] += 0.5
        M2[m, k * N + i] += 0.5
        M2[286 + _tri_idx(i, j), i * N + k] -= 0.5
        M2[286 + _tri_idx(i, j), k * N + i] -= 0.5
        M2[286 + _tri_idx(k, j), i * N + k] -= 0.5
        M2[286 + _tri_idx(k, j), k * N + i] -= 0.5
    for i in range(N):
        for j in range(i + 1):
            M2[286 + _tri_idx(i, j), i * N + i] += 1.0
    return M1, M2


# contraction split of the 364 pre-square rows for the post matmul
KSPLIT = (128, 128, 108)


def build_nc(b_shard):
    """Build + compile the per-core Bass program for a batch shard of b_shard."""
    assert b_shard % CHUNK == 0
    nchunk = b_shard // CHUNK
    Relu = mybir.ActivationFunctionType.Relu
    Exp = mybir.ActivationFunctionType.Exp
    Ln = mybir.ActivationFunctionType.Ln
    add_op = mybir.AluOpType.add
    max_op = mybir.AluOpType.max

    nc = bacc.Bacc("TRN2", target_bir_lowering=False, debug=False)

    xT = nc.dram_tensor("xT", (N, b_shard), F32, kind="ExternalInput").ap()
    W1 = nc.dram_tensor("W1", (N, HID), F32, kind="ExternalInput").ap()
    b1 = nc.dram_tensor("b1", (HID,), F32, kind="ExternalInput").ap()
    W2 = nc.dram_tensor("W2", (HID, HID), F32, kind="ExternalInput").ap()
    b2 = nc.dram_tensor("b2", (HID,), F32, kind="ExternalInput").ap()
    W3 = nc.dram_tensor("W3", (HID, TRI + 1), F32, kind="ExternalInput").ap()
    b3 = nc.dram_tensor("b3", (TRI + 1,), F32, kind="ExternalInput").ap()
    M1d = nc.dram_tensor("M1", (TRI, NPRE), F32, kind="ExternalInput").ap()
    M2d = nc.dram_tensor("M2", (NPRE, 144), F32, kind="ExternalInput").ap()
    mmt = nc.dram_tensor("mmt", (b_shard, 144), F32, kind="ExternalOutput").ap()
    cout = nc.dram_tensor("c", (b_shard,), F32, kind="ExternalOutput").ap()

    with ExitStack() as ctx:
        tc = ctx.enter_context(tile.TileContext(nc))
        const = ctx.enter_context(tc.tile_pool(name="const", bufs=1))

        w1sb = const.tile([N, HID], F32)
        nc.sync.dma_start(w1sb[:], W1)
        w2sb = []
        for k in range(4):
            t = const.tile([128, HID], F32, name=f"w2_{k}")
            nc.sync.dma_start(t[:], W2[k * 128:(k + 1) * 128, :])
            w2sb.append(t)
        w3sb = []
        for k in range(4):
            t = const.tile([128, TRI + 1], F32, name=f"w3_{k}")
            nc.sync.dma_start(t[:], W3[k * 128:(k + 1) * 128, :])
            w3sb.append(t)
        m1sb = const.tile([TRI, NPRE], F32)
        nc.sync.dma_start(m1sb[:], M1d)
        m2sb = []
        koff = 0
        for k, ksz in enumerate(KSPLIT):
            t = const.tile([ksz, 144], F32, name=f"m2_{k}")
            nc.sync.dma_start(t[:], M2d[koff:koff + ksz, :])
            m2sb.append(t)
            koff += ksz
        b1sb = const.tile([128, 4], F32)
        nc.sync.dma_start(b1sb[:], b1.rearrange("(m p) -> p m", p=128))
        b2sb = const.tile([128, 4], F32)
        nc.sync.dma_start(b2sb[:], b2.rearrange("(m p) -> p m", p=128))
        b3sb = const.tile([TRI + 1, 1], F32)
        nc.sync.dma_start(b3sb[:], b3.rearrange("(t o) -> t o", o=1))

        sb = ctx.enter_context(tc.tile_pool(name="sb", bufs=3))
        # PSUM: 8 banks total; each tag below gets bufs slots of 1 bank each
        psA = ctx.enter_context(tc.tile_pool(name="psA", bufs=2, space="PSUM"))
        psB = ctx.enter_context(tc.tile_pool(name="psB", bufs=2, space="PSUM"))
        psC = ctx.enter_context(tc.tile_pool(name="psC", bufs=2, space="PSUM"))
        psD = ctx.enter_context(tc.tile_pool(name="psD", bufs=2, space="PSUM"))

        cout2d = cout.rearrange("(o n) -> o n", o=1)

        for c in range(nchunk):
            cs = slice(c * CHUNK, (c + 1) * CHUNK)
            xt = sb.tile([N, CHUNK], F32, tag="xt", bufs=3)
            nc.sync.dma_start(xt[:], xT[:, cs])

            # ---- layer 1: h1^T [512, CHUNK] ----
            h1 = []
            for m in range(4):
                hp = psA.tile([128, CHUNK], F32, tag="h1ps")
                nc.tensor.matmul(hp[:], w1sb[:, m * 128:(m + 1) * 128], xt[:],
                                 start=True, stop=True)
                hs = sb.tile([128, CHUNK], F32, tag="h1sb", bufs=8)
                nc.vector.tensor_scalar(hs[:], hp[:], b1sb[:, m:m + 1], 0.0,
                                        add_op, max_op)
                h1.append(hs)

            # ---- layer 2: h2^T [512, CHUNK] ----
            h2 = []
            for j in range(4):
                hp = psB.tile([128, CHUNK], F32, tag="h2ps")
                for k in range(4):
                    nc.tensor.matmul(hp[:], w2sb[k][:, j * 128:(j + 1) * 128],
                                     h1[k][:], start=(k == 0), stop=(k == 3))
                hs = sb.tile([128, CHUNK], F32, tag="h2sb", bufs=8)
                nc.vector.tensor_scalar(hs[:], hp[:], b2sb[:, j:j + 1], 0.0,
                                        add_op, max_op)
                h2.append(hs)

            # ---- layer 3 + softplus: tri [79, CHUNK] ----
            tp = psC.tile([128, CHUNK], F32, tag="typs")
            for k in range(4):
                nc.tensor.matmul(tp[:TRI + 1], w3sb[k][:], h2[k][:],
                                 start=(k == 0), stop=(k == 3))
            esb = sb.tile([TRI + 1, CHUNK], F32, tag="esb", bufs=2)
            nc.scalar.activation(esb[:], tp[:TRI + 1], Exp, bias=b3sb[:])
            trisb = sb.tile([TRI + 1, CHUNK], F32, tag="trisb", bufs=2)
            nc.scalar.activation(trisb[:], esb[:], Ln, bias=1.0)
            nc.sync.dma_start(cout2d[:, cs], trisb[TRI:TRI + 1, :])

            # ---- quad pre-map + square: s [364, CHUNK] ----
            ssb = []
            moff = 0
            for m, msz in enumerate(KSPLIT):
                yp = psC.tile([128, CHUNK], F32, tag="typs")
                nc.tensor.matmul(yp[:msz], m1sb[:, moff:moff + msz],
                                 trisb[0:TRI, :], start=True, stop=True)
                sq = sb.tile([128, CHUNK], F32, tag="ssb", bufs=6)
                nc.scalar.square(sq[:msz], yp[:msz])
                ssb.append(sq)
                moff += msz

            # ---- quad post-map, batch-major out [128b, 144] ----
            for bt in range(4):
                op = psD.tile([128, 144], F32, tag="ops")
                for k, ksz in enumerate(KSPLIT):
                    nc.tensor.matmul(op[:], ssb[k][:ksz, bt * 128:(bt + 1) * 128],
                                     m2sb[k][:], start=(k == 0), stop=(k == 2))
                ob = sb.tile([128, 144], F32, tag="ob", bufs=4)
                nc.vector.tensor_copy(ob[:], op[:])
                nc.sync.dma_start(mmt[c * CHUNK + bt * 128:
                                      c * CHUNK + (bt + 1) * 128, :], ob[:])

    nc.compile()
    return nc


_NC_CACHE = {}
LAST_RESULT = None


def _get_nc(b_shard):
    if b_shard not in _NC_CACHE:
        _NC_CACHE[b_shard] = build_nc(b_shard)
    return _NC_CACHE[b_shard]


def kernel(x, W1, b1, W2, b2, W3, b3, **run_kwargs):
    x = np.ascontiguousarray(x, np.float32)
    B = x.shape[0]
    assert B % N_CORES == 0
    b_shard = B // N_CORES
    nc = _get_nc(b_shard)

    M1, M2 = make_quad_maps()
    xT = np.ascontiguousarray(x.T)
    common = {
        "W1": np.ascontiguousarray(W1, np.float32),
        "b1": np.ascontiguousarray(b1, np.float32),
        "W2": np.ascontiguousarray(W2, np.float32),
        "b2": np.ascontiguousarray(b2, np.float32),
        "W3": np.ascontiguousarray(W3, np.float32),
        "b3": np.ascontiguousarray(b3, np.float32),
        "M1": M1,
        "M2": M2,
    }
    in_maps = [
        {"xT": np.ascontiguousarray(xT[:, s * b_shard:(s + 1) * b_shard]), **common}
        for s in range(N_CORES)
    ]
    res = run_bass_kernel_spmd(nc, in_maps, core_ids=list(range(N_CORES)),
                               **run_kwargs)
    global LAST_RESULT
    LAST_RESULT = res
    mmt = np.concatenate([r["mmt"] for r in res.results], axis=0)
    c = np.concatenate([r["c"] for r in res.results], axis=0)
    return mmt.reshape(B, N, N), c


if __name__ == "__main__":
    rng = np.random.default_rng(0)
    B = 4096 * 8
    inputs = {
        "x": rng.standard_normal((B, N), dtype=np.float32),
        "W1": (rng.standard_normal((N, HID), dtype=np.float32) / np.sqrt(N)),
        "b1": np.zeros(HID, np.float32),
        "W2": (rng.standard_normal((HID, HID), dtype=np.float32) / np.sqrt(HID)),
        "b2": np.zeros(HID, np.float32),
        "W3": (rng.standard_normal((HID, TRI + 1), dtype=np.float32) / np.sqrt(HID)),
        "b3": np.zeros(TRI + 1, np.float32),
    }
    mmt, c = kernel(**inputs)
    print(mmt.shape, c.shape, mmt.dtype)


# revision 17
# speedup vs baseline: 9905.9182x; 9905.9182x over previous
"""Trainium2 Bass kernel for CholeskyPlusConst MLP.

Computes, for x [B, 12]:
    h1  = relu(x @ W1 + b1)            [B, 512]
    h2  = relu(h1 @ W2 + b2)           [B, 512]
    out = softplus(h2 @ W3 + b3)       [B, 79]
    L   = tril-scatter(out[:, :78])    [B, 12, 12]
    MMT = L @ L^T                      [B, 12, 12]
    c   = out[:, 78]                   [B]
returns (MMT, c).

Strategy: pure data parallel over 8 NeuronCores (batch sharded, weights
replicated). On-core the pipeline runs feature-major (features on SBUF
partitions, batch on the free dim, 512-wide batch chunks) so every layer
is a plain stationary-weight matmul. Matmuls use float32r (TF32-like,
single-pass on the PE — ~3x faster than fp32, ~15x more accurate than
bf16). The quadratic L@L^T step uses a*b = ((a+b)^2 - a^2 - b^2)/2:
a constant 0/1 pre-map M1 (78 -> 364) on the tensor engine, an
elementwise square, and a constant post-map M2 (364 -> 144) whose matmul
takes the squared data as the *stationary* operand so the result lands
batch-major [128b, 144] and is DMA'd out contiguously with no transpose.
Softplus = Ln(Exp(x)+1) on ScalarE (both in one activation-table set).
"""

import numpy as np
from contextlib import ExitStack

import concourse.bass as bass
import concourse.mybir as mybir
import concourse.tile as tile
from concourse import bacc
from concourse.bass_utils import run_bass_kernel_spmd

N = 12
TRI = 78            # N*(N+1)//2
HID = 512
NPRE = 286 + TRI    # 364 pre-square rows
B_FULL = 262144
N_CORES = 8
CHUNK = 512         # batch columns per pipeline chunk (one PSUM bank fp32)

F32 = mybir.dt.float32
F32R = mybir.dt.float32r
F16 = mybir.dt.float16

# All ScalarE functions this kernel uses live in one activation-table set;
# starve the other sets of them so the table-load pass picks the combined
# set and the table is loaded exactly once (the greedy chooser otherwise
# alternates exp_and_others <-> natural_log every chunk, ~2.6us a flip).
_ACT_SET = "natural_log_exp_and_others"
_ACT_FNS = {
    mybir.ActivationFunctionType.Exp,
    mybir.ActivationFunctionType.Ln,
    mybir.ActivationFunctionType.Square,
    mybir.ActivationFunctionType.Relu,
    mybir.ActivationFunctionType.Copy,
    mybir.ActivationFunctionType.Identity,
}
_tables_patched = False


def _patch_act_tables():
    global _tables_patched
    if _tables_patched:
        return
    orig = bacc.get_activation_tables

    def patched(arch):
        tabs = orig(arch)
        out = {}
        for name, fns in tabs.items():
            out[name] = set(fns) if name == _ACT_SET else set(fns) - _ACT_FNS
        return out

    bacc.get_activation_tables = patched
    _tables_patched = True


def _round_f32r(a):
    """Round-to-nearest-even to float32r (13 low mantissa bits dropped)."""
    u = np.ascontiguousarray(a, np.float32).view(np.uint32)
    r = (u + 0x0FFF + ((u >> 13) & 1)) & np.uint32(0xFFFFE000)
    return r.view(np.float32).copy()


def _tri_idx(i, j):
    return i * (i + 1) // 2 + j


def make_quad_maps():
    """M1 [78, 364] pre-square map and M2 [364, 144] post-square map with
    MMT.reshape(B,144) == ((tri @ M1)**2) @ M2."""
    triples = [(i, k, j) for i in range(N) for k in range(i) for j in range(k + 1)]
    M1 = np.zeros((TRI, NPRE), np.float32)
    for m, (i, k, j) in enumerate(triples):
        M1[_tri_idx(i, j), m] += 1
        M1[_tri_idx(k, j), m] += 1
    for t in range(TRI):
        M1[t, 286 + t] = 1
    M2 = np.zeros((NPRE, 144), np.float32)
    for m, (i, k, j) in enumerate(triples):
        M2[m, i * N + k] += 0.5
        M2[m, k * N + i] += 0.5
        M2[286 + _tri_idx(i, j), i * N + k] -= 0.5
        M2[286 + _tri_idx(i, j), k * N + i] -= 0.5
        M2[286 + _tri_idx(k, j), i * N + k] -= 0.5
        M2[286 + _tri_idx(k, j), k * N + i] -= 0.5
    for i in range(N):
        for j in range(i + 1):
            M2[286 + _tri_idx(i, j), i * N + i] += 1.0
    return M1, M2


# contraction split of the 364 pre-square rows for the post matmul
KSPLIT = (128, 128, 108)


def build_nc(b_shard):
    """Build + compile the per-core Bass program for a batch shard of b_shard."""
    assert b_shard % CHUNK == 0
    _patch_act_tables()
    nchunk = b_shard // CHUNK
    Exp = mybir.ActivationFunctionType.Exp
    Relu = mybir.ActivationFunctionType.Relu
    Ln = mybir.ActivationFunctionType.Ln
    add_op = mybir.AluOpType.add
    max_op = mybir.AluOpType.max

    nc = bacc.Bacc("TRN2", target_bir_lowering=False, debug=False)

    xT = nc.dram_tensor("xT", (N, b_shard), F16, kind="ExternalInput").ap()
    W1 = nc.dram_tensor("W1", (N, HID), F16, kind="ExternalInput").ap()
    b1 = nc.dram_tensor("b1", (HID,), F32, kind="ExternalInput").ap()
    W2 = nc.dram_tensor("W2", (HID, HID), F32R, kind="ExternalInput").ap()
    b2 = nc.dram_tensor("b2", (HID,), F32, kind="ExternalInput").ap()
    W3 = nc.dram_tensor("W3", (HID, TRI + 1), F32R, kind="ExternalInput").ap()
    b3 = nc.dram_tensor("b3", (TRI + 1,), F32, kind="ExternalInput").ap()
    M1d = nc.dram_tensor("M1", (TRI, NPRE), F32R, kind="ExternalInput").ap()
    M2d = nc.dram_tensor("M2", (NPRE, 144), F16, kind="ExternalInput").ap()
    mmt = nc.dram_tensor("mmt", (b_shard, 144), F32, kind="ExternalOutput").ap()
    cout = nc.dram_tensor("c", (b_shard,), F32, kind="ExternalOutput").ap()

    with ExitStack() as ctx:
        tc = ctx.enter_context(tile.TileContext(nc))
        const = ctx.enter_context(tc.tile_pool(name="const", bufs=1))

        # W1 replicated on partition rows 0-11 and 32-43 so two L1 matmuls
        # can run concurrently in distinct PE row-groups (tile_position).
        w1sb = const.tile([32 + N, HID], F16)
        nc.gpsimd.dma_start(w1sb[0:N, :], W1)
        nc.gpsimd.dma_start(w1sb[32:32 + N, :], W1)
        w2sb = []
        for k in range(4):
            t = const.tile([128, HID], F32R, name=f"w2_{k}")
            nc.gpsimd.dma_start(t[:], W2[k * 128:(k + 1) * 128, :])
            w2sb.append(t)
        w3sb = []
        for k in range(4):
            t = const.tile([128, TRI + 1], F32R, name=f"w3_{k}")
            nc.gpsimd.dma_start(t[:], W3[k * 128:(k + 1) * 128, :])
            w3sb.append(t)
        m1sb = const.tile([TRI, NPRE], F32R)
        nc.gpsimd.dma_start(m1sb[:], M1d)
        m2sb = []
        koff = 0
        for k, ksz in enumerate(KSPLIT):
            t = const.tile([ksz, 144], F16, name=f"m2_{k}")
            nc.gpsimd.dma_start(t[:], M2d[koff:koff + ksz, :])
            m2sb.append(t)
            koff += ksz
        b1sb = const.tile([128, 4], F32)
        nc.gpsimd.dma_start(b1sb[:], b1.rearrange("(m p) -> p m", p=128))
        b2sb = const.tile([128, 4], F32)
        nc.gpsimd.dma_start(b2sb[:], b2.rearrange("(m p) -> p m", p=128))
        b3sb = const.tile([TRI + 1, 1], F32)
        nc.gpsimd.dma_start(b3sb[:], b3.rearrange("(t o) -> t o", o=1))

        sb = ctx.enter_context(tc.tile_pool(name="sb", bufs=3))
        # PSUM: 8 banks total; each tag below gets bufs slots of 1 bank each
        psA = ctx.enter_context(tc.tile_pool(name="psA", bufs=4, space="PSUM"))
        psB = ctx.enter_context(tc.tile_pool(name="psB", bufs=2, space="PSUM"))
        psC = ctx.enter_context(tc.tile_pool(name="psC", bufs=2, space="PSUM"))

        cout2d = cout.rearrange("(o n) -> o n", o=1)

        for c in range(nchunk):
            cs = slice(c * CHUNK, (c + 1) * CHUNK)
            xt = sb.tile([32 + N, CHUNK], F16, tag="xt", bufs=4)
            nc.sync.dma_start(xt[0:N, :], xT[:, cs])
            nc.sync.dma_start(xt[32:32 + N, :], xT[:, cs])

            # ---- layer 1: h1^T [512, CHUNK], two row-groups in parallel ----
            h1 = []
            for pair in range(2):
                hps = []
                for g in range(2):
                    m = pair * 2 + g
                    hp = psA.tile([128, CHUNK], F32, tag="h1ps")
                    nc.tensor.matmul(hp[:],
                                     w1sb[32 * g:32 * g + N,
                                          m * 128:(m + 1) * 128],
                                     xt[32 * g:32 * g + N, :],
                                     start=True, stop=True)
                    hps.append(hp)
                for g in range(2):
                    m = pair * 2 + g
                    hs = sb.tile([128, CHUNK], F32R, tag="h1sb", bufs=8)
                    nc.vector.tensor_scalar(hs[:], hps[g][:], b1sb[:, m:m + 1],
                                            0.0, add_op, max_op)
                    h1.append(hs)

            # ---- layer 2: h2^T [512, CHUNK] ----
            h2 = []
            for j in range(4):
                hp = psB.tile([128, CHUNK], F32, tag="h2ps")
                for k in range(4):
                    nc.tensor.matmul(hp[:], w2sb[k][:, j * 128:(j + 1) * 128],
                                     h1[k][:], start=(k == 0), stop=(k == 3))
                hs = sb.tile([128, CHUNK], F32R, tag="h2sb", bufs=8)
                if j < 2:
                    nc.scalar.activation(hs[:], hp[:], Relu, bias=b2sb[:, j:j + 1])
                else:
                    nc.vector.tensor_scalar(hs[:], hp[:], b2sb[:, j:j + 1], 0.0,
                                            add_op, max_op)
                h2.append(hs)

            # ---- layer 3 + softplus: tri [79, CHUNK] ----
            tp = psC.tile([128, CHUNK], F32, tag="typs")
            for k in range(4):
                nc.tensor.matmul(tp[:TRI + 1], w3sb[k][:], h2[k][:],
                                 start=(k == 0), stop=(k == 3))
            esb = sb.tile([TRI + 1, CHUNK], F32, tag="esb", bufs=3)
            nc.scalar.activation(esb[:], tp[:TRI + 1], Exp, bias=b3sb[:])
            trisb = sb.tile([TRI + 1, CHUNK], F32R, tag="trisb", bufs=3)
            nc.scalar.activation(trisb[:], esb[:], Ln, bias=1.0)
            nc.sync.dma_start(cout2d[:, cs], trisb[TRI:TRI + 1, :].bitcast(F32))

            # ---- quad pre-map + square: s [364, CHUNK] ----
            ssb = []
            moff = 0
            for m, msz in enumerate(KSPLIT):
                yp = psC.tile([128, CHUNK], F32, tag="typs")
                nc.tensor.matmul(yp[:msz], m1sb[:, moff:moff + msz],
                                 trisb[0:TRI, :], start=True, stop=True)
                sq = sb.tile([128, CHUNK], F16, tag="ssb", bufs=6)
                nc.scalar.square(sq[:msz], yp[:msz])
                ssb.append(sq)
                moff += msz

            # ---- quad post-map, batch-major out [128b, 144] ----
            ob = sb.tile([128, 4, 144], F32, tag="ob", bufs=3)
            for bt in range(4):
                op = psC.tile([128, CHUNK], F32, tag="typs", name="ops")[:, :144]
                for k, ksz in enumerate(KSPLIT):
                    nc.tensor.matmul(op[:], ssb[k][:ksz, bt * 128:(bt + 1) * 128],
                                     m2sb[k][:], start=(k == 0), stop=(k == 2))
                nc.vector.tensor_copy(ob[:, bt, :], op[:])
            nc.sync.dma_start(
                mmt[cs, :].rearrange("(bt p) f -> p bt f", p=128), ob[:])

    nc.compile()
    return nc


_NC_CACHE = {}
LAST_RESULT = None


def _get_nc(b_shard):
    if b_shard not in _NC_CACHE:
        _NC_CACHE[b_shard] = build_nc(b_shard)
    return _NC_CACHE[b_shard]


def kernel(x, W1, b1, W2, b2, W3, b3, **run_kwargs):
    x = np.ascontiguousarray(x, np.float32)
    B = x.shape[0]
    assert B % N_CORES == 0
    b_shard = B // N_CORES
    nc = _get_nc(b_shard)

    M1, M2 = make_quad_maps()
    xT = np.ascontiguousarray(x, np.float32).T.astype(np.float16)
    common = {
        "W1": np.ascontiguousarray(W1, np.float16),
        "b1": np.ascontiguousarray(b1, np.float32),
        "W2": _round_f32r(W2),
        "b2": np.ascontiguousarray(b2, np.float32),
        "W3": _round_f32r(W3),
        "b3": np.ascontiguousarray(b3, np.float32),
        "M1": M1,
        "M2": M2.astype(np.float16),
    }
    in_maps = [
        {"xT": np.ascontiguousarray(xT[:, s * b_shard:(s + 1) * b_shard]), **common}
        for s in range(N_CORES)
    ]
    res = run_bass_kernel_spmd(nc, in_maps, core_ids=list(range(N_CORES)),
                               **run_kwargs)
    global LAST_RESULT
    LAST_RESULT = res
    mmt = np.concatenate([r["mmt"] for r in res.results], axis=0)
    c = np.concatenate([r["c"] for r in res.results], axis=0)
    return mmt.reshape(B, N, N), c


if __name__ == "__main__":
    rng = np.random.default_rng(0)
    B = 4096 * 8
    inputs = {
        "x": rng.standard_normal((B, N), dtype=np.float32),
        "W1": (rng.standard_normal((N, HID), dtype=np.float32) / np.sqrt(N)),
        "b1": np.zeros(HID, np.float32),
        "W2": (rng.standard_normal((HID, HID), dtype=np.float32) / np.sqrt(HID)),
        "b2": np.zeros(HID, np.float32),
        "W3": (rng.standard_normal((HID, TRI + 1), dtype=np.float32) / np.sqrt(HID)),
        "b3": np.zeros(TRI + 1, np.float32),
    }
    mmt, c = kernel(**inputs)
    print(mmt.shape, c.shape, mmt.dtype)
